# revision 23
# baseline (speedup 1.0000x reference)
"""Fused attention kernel for TRN2, data-parallel over 8 NeuronCores.

Problem: LN -> qk/v projections -> softplus-polar embedding -> attention
-> output projection.  B=8 batch elements are sharded one-per-core; each
core runs an identical single-core program (no collectives).

Layout strategy (per core, N=1024 tokens, D=1024, H=16 heads, DH=64):
  - LN in [n, d] layout (free-axis reductions, bn_stats), PE-transpose
    via a bf16 identity matmul to xnT [d, n] bf16.
  - q/k produced TRANSPOSED ([e, n]) with w_qk tiles as the stationary
    operand; v produced natural ([n, e]) with xnT as stationary.
  - polar: softplus = Ln(1+Exp(x)) on ScalarE (Exp in-place on PSUM; ACT
    ops batched [Exp,Exp,Ln,Ln] per step), the 64-row head slice is
    duplicated into both partition halves by two SBUF->SBUF DMAs, then
    one DVE multiply against a [cosT; sinT] table.
  - attention entirely in transposed layout: S^T = k2t.T @ q2t (K=128,
    single k-tile), exp on ScalarE (scale=DH^-0.5 fused) from PSUM,
    O_un^T = V'.T @ E^T in two n-halves ([65, 512] psum, 1 bank each);
    V' carries a ones-column at col 64 so the softmax denominator L
    lands on psum row 64 for every head.
  - normalize: per half, DVE drains psum rows 0:64 -> po_sb (bf16) and
    row 64 -> the pair tile llp (even head at partition 64, odd head at
    partition 96 -- engine writes must start at partition 0/32/64/96).
    One reciprocal_approx_fast over llp[64:97] covers both heads; a DRAM
    round trip broadcasts each 1/L row to [128, N] and one DVE multiply
    per head writes the normalized O^T into ot_sb.
  - ScalarE activation tables: Exp and Ln are forced into the shared
    `natural_log_exp_and_others` set (see _patched_tables) so the main
    loop never reloads activation tables.
  - final GEMM: lhsT = O^T tiles, rhs = w_out; b_out enters as a DVE
    broadcast add during the PSUM drain (no K=1 bias matmuls).

ln_gamma is folded into w_qk/w_v rows on the host; ln_beta enters as an
ACT bias ([128,1] per e-chunk) for q/k; the v-path beta term (beta@w_v)
commutes through attention (softmax rows sum to 1) and is folded into
b_out on the host: b_out' = b_out + (beta @ w_v) @ w_out.
"""

import os

import ml_dtypes
import numpy as np

import concourse.bass as bass
import concourse.tile as tile
from concourse import bacc, mybir
from concourse.bass_utils import run_bass_kernel_spmd
from concourse.bass import _add_dep_helper
from concourse.masks import make_identity

# ---- force Exp and Ln into one activation-table set -----------------------
# The default greedy assignment puts Exp in `exp_and_others` and Ln in
# `natural_log`, inserting a ~1.3us ACT_TABLE_LOAD at every Exp<->Ln
# transition (2 per main-loop step).  Emptying those two sets (names and
# positions preserved, so act_func_set_id indices stay valid) makes both
# resolve to `natural_log_exp_and_others`.
import concourse.hw_specs as _hw_specs
import concourse.bacc as _bacc_mod

_orig_get_tables = _hw_specs.get_activation_tables


def _patched_tables(arch):
    out = {}
    for name, fns in _orig_get_tables(arch).items():
        if name in ("exp_and_others", "natural_log"):
            out[name] = set()
        else:
            out[name] = fns
    return out


F32 = mybir.dt.float32
BF16 = mybir.dt.bfloat16
AF = mybir.ActivationFunctionType
ALU = mybir.AluOpType

B, N, D, H, DH = 8, 1024, 1024, 16, 64
NC_, DT_, EC_Q, MC_ = 8, 8, 8, 8  # n-chunks, d-tiles, q e-chunks, m-tiles
SCALE = DH ** -0.5


def _emit(tc):
    nc = tc.nc

    x_d = nc.dram_tensor("x", [N, D], F32, kind="ExternalInput").ap()
    wqk_d = nc.dram_tensor("wqk", [D, 2 * H * DH], BF16, kind="ExternalInput").ap()
    wv_d = nc.dram_tensor("wv", [D, H * DH], BF16, kind="ExternalInput").ap()
    wout_d = nc.dram_tensor("wout", [H * DH, D], BF16, kind="ExternalInput").ap()
    csq_d = nc.dram_tensor("csq", [128, N], BF16, kind="ExternalInput").ap()
    csk_d = nc.dram_tensor("csk", [128, N], BF16, kind="ExternalInput").ap()
    qbias_d = nc.dram_tensor("qbias", [128, 16], F32, kind="ExternalInput").ap()
    bout_d = nc.dram_tensor("bout", [1, D], F32, kind="ExternalInput").ap()
    out_d = nc.dram_tensor("out", [N, D], F32, kind="ExternalOutput").ap()

    def bcast(ap_1xN, parts=128):
        return bass.AP(
            tensor=ap_1xN.tensor, offset=ap_1xN.offset, ap=[[0, parts]] + ap_1xN.ap[1:]
        )

    with (
        tc.tile_pool(name="const", bufs=1) as const,
        tc.tile_pool(name="xin", bufs=4) as xin,
        tc.tile_pool(name="ln", bufs=3) as ln,
        tc.tile_pool(name="xnbfp", bufs=3) as xnbfp,
        tc.tile_pool(name="wqs", bufs=2) as wqs,
        tc.tile_pool(name="spp", bufs=2) as spp,
        tc.tile_pool(name="q2p", bufs=4) as q2p,
        tc.tile_pool(name="k2p", bufs=4) as k2p,
        tc.tile_pool(name="etp", bufs=18) as etp,
        tc.tile_pool(name="reclp", bufs=3) as reclp,
        tc.tile_pool(name="llp", bufs=2) as llp,
        tc.tile_pool(name="lbcp", bufs=3) as lbcp,
        tc.tile_pool(name="drsp", bufs=4, space="DRAM") as drsp,
        tc.tile_pool(name="outp", bufs=2) as outp,
        tc.tile_pool(name="psA", bufs=3, space="PSUM") as psA,
        tc.tile_pool(name="psO", bufs=2, space="PSUM") as psO,
    ):
        # ---- resident constants -------------------------------------
        # x prefetch first: the LN chain is the critical path at startup.
        x_tiles = {}

        def fetch_x(c):
            if c < NC_ and c not in x_tiles:
                t = xin.tile([128, D], F32, tag="x")
                nc.sync.dma_start(out=t[:], in_=x_d[c * 128 : (c + 1) * 128, :])
                x_tiles[c] = t

        for c in range(3):
            fetch_x(c)

        # wv arrives piecewise: v-proj's t-th matmul only needs piece t,
        # so early pieces unblock chunk-0 v work while the rest stream in.
        # The scalar queue carries no DMAs (it must reach the LN Sqrts
        # quickly); wout is issued at the end of Phase A emission.
        wv_sb = const.tile([128, DT_, 1024], BF16, tag="wv")
        wv_r = wv_d.rearrange("(t p) e -> p t e", p=128)
        nc.gpsimd.dma_start(out=wv_sb[:, 0:4, :], in_=wv_r[:, 0:4, :])
        nc.sync.dma_start(out=wv_sb[:, 4:8, :], in_=wv_r[:, 4:8, :])
        csq_sb = const.tile([128, N], BF16, tag="csq")
        nc.sync.dma_start(out=csq_sb[:], in_=csq_d)
        csk_sb = const.tile([128, N], BF16, tag="csk")
        nc.sync.dma_start(out=csk_sb[:], in_=csk_d)
        qbias_sb = const.tile([128, 16], F32, tag="qbias")
        nc.sync.dma_start(out=qbias_sb[:], in_=qbias_d)
        bout_bc = const.tile([128, 1024], F32, tag="boutbc")
        nc.gpsimd.dma_start(out=bout_bc[:], in_=bcast(bout_d))
        eps_sb = const.tile([128, 1], F32, tag="eps")
        nc.vector.memset(eps_sb[:], 1e-5)
        ident = const.tile([128, 128], BF16, tag="ident")
        make_identity(nc, ident[:])

        xnT = const.tile([128, DT_, N], BF16, tag="xnT")
        # V' per m-chunk: [m-part, chunk, head*65]; col 64 of each head
        # block is the ones-column (softmax denominator row).
        vp = const.tile([128, MC_, H * 65], BF16, tag="vp")
        nc.gpsimd.memset(
            vp.rearrange("p m (h w) -> p m h w", w=65)[:, :, :, 64:65], 1.0
        )
        ot_sb = const.tile([128, DT_, N], BF16, tag="otsb")

        # ---- Phase A: layernorm + PE transpose + v projection -------
        ln_state = {}
        xnbf_tiles = {}

        def ln_stats(c):
            if c >= NC_ or c in ln_state:
                return
            x_t = x_tiles[c]
            st = ln.tile([128, 2, 6], F32, tag="st")
            for s in range(2):
                nc.vector.bn_stats(out=st[:, s, :], in_=x_t[:, s * 512 : (s + 1) * 512])
            mv = ln.tile([128, 2], F32, tag="mv")
            nc.vector.bn_aggr(out=mv[:], in_=st[:])
            rsig = ln.tile([128, 1], F32, tag="rsig")
            # rsqrt via exp(-0.5*ln(var+eps)): keeps ScalarE on the one
            # shared exp/ln table set (no Sqrt-table load mid-kernel).
            nc.scalar.activation(rsig[:], mv[:, 1:2], AF.Ln, bias=eps_sb[:])
            nc.scalar.activation(rsig[:], rsig[:], AF.Exp, scale=-0.5)
            ln_state[c] = (mv, rsig)

        def ln_ts(c):
            if c >= NC_ or c in xnbf_tiles:
                return
            mv, rsig = ln_state.pop(c)
            xnbf = xnbfp.tile([128, D], BF16, tag="xnbf")
            nc.vector.tensor_scalar(
                out=xnbf[:],
                in0=x_tiles[c],
                scalar1=mv[:, 0:1],
                scalar2=rsig[:],
                op0=ALU.subtract,
                op1=ALU.mult,
            )
            xnbf_tiles[c] = xnbf

        def v_proj(c):
            psv = psA.tile([128, N], F32, tag="ps")
            for t in range(DT_):
                for hlf in range(2):
                    nc.tensor.matmul(
                        psv[:, hlf * 512 : (hlf + 1) * 512],
                        lhsT=xnT[:, t, c * 128 : (c + 1) * 128],
                        rhs=wv_sb[:, t, hlf * 512 : (hlf + 1) * 512],
                        start=(t == 0),
                        stop=(t == DT_ - 1),
                    )
            nc.vector.tensor_copy(
                out=vp.rearrange("p m (h w) -> p m h w", w=65)[:, c, :, 0:64],
                in_=psv.rearrange("p (h w) -> p h w", w=64),
            )

        ln_stats(0)
        ln_ts(0)
        ln_stats(1)
        ln_ts(1)
        ln_stats(2)
        ln_ts(2)
        # v projection runs one chunk behind the transposes so the xnT
        # psum->SBUF drain latency never sits on the PE critical path;
        # v(7) is emitted after qk(0)'s matmuls to fill the PE while the
        # first softplus/polar chain runs.
        for c in range(NC_):
            xnbf = xnbf_tiles.pop(c)
            pst = psA.tile([128, N], F32, tag="ps")
            for t in range(DT_):
                nc.tensor.matmul(
                    pst[:, t * 128 : (t + 1) * 128],
                    lhsT=xnbf[:, t * 128 : (t + 1) * 128],
                    rhs=ident[:],
                    start=True,
                    stop=True,
                )
            nc.vector.tensor_copy(
                out=xnT[:, :, c * 128 : (c + 1) * 128],
                in_=pst.rearrange("p (t n) -> p t n", n=128),
            )
            ln_ts(c + 2)
            fetch_x(c + 3)
            if c > 0:
                v_proj(c - 1)
            ln_stats(c + 3)

        wout_sb = const.tile([128, DT_, 1024], BF16, tag="wout")
        wout_r = wout_d.rearrange("(t p) e -> p t e", p=128)
        nc.gpsimd.dma_start(out=wout_sb[:, 0:4, :], in_=wout_r[:, 0:4, :])
        nc.sync.dma_start(out=wout_sb[:, 4:8, :], in_=wout_r[:, 4:8, :])

        # ---- helpers ------------------------------------------------
        def qk_mms(j):
            psqk = []
            for is_q in (True, False):
                ecol = j * 128 if is_q else 1024 + j * 128
                wt = wqs.tile([128, DT_, 128], BF16, tag="wt")
                nc.sync.dma_start(
                    out=wt[:],
                    in_=wqk_d.rearrange("(t p) e -> p t e", p=128)[
                        :, :, ecol : ecol + 128
                    ],
                )
                ps = psA.tile([128, N], F32, tag="ps")
                for t in range(DT_):
                    for hlf in range(2):
                        nc.tensor.matmul(
                            ps[:, hlf * 512 : (hlf + 1) * 512],
                            lhsT=wt[:, t, :],
                            rhs=xnT[:, t, hlf * 512 : (hlf + 1) * 512],
                            start=(t == 0),
                            stop=(t == DT_ - 1),
                        )
                psqk.append(ps)
            return psqk

        def qk_acts(j, psqk):
            # Exp/Ln share one table set, so interleave per operand:
            # Exp_q,Ln_q run as soon as q's 8 matmuls land (not after all
            # 16), and the q psum frees a rotation slot earlier.
            sps = []
            for is_q, ps in zip((True, False), psqk):
                bcol = j if is_q else 8 + j
                nc.scalar.activation(
                    ps[:], ps[:], AF.Exp, bias=qbias_sb[:, bcol : bcol + 1]
                )
                sp = spp.tile([128, N], BF16, tag="sp")
                nc.scalar.activation(sp[:], ps[:], AF.Ln, bias=1.0)
                sps.append(sp)
            out = []
            for is_q, sp in zip((True, False), sps):
                pool = q2p if is_q else k2p
                cs = csq_sb if is_q else csk_sb
                tiles = []
                for hh in range(2):
                    dup = pool.tile([128, N], BF16, tag="d")
                    nc.sync.dma_start(
                        out=dup[0:64, :], in_=sp[hh * 64 : hh * 64 + 64, :]
                    )
                    nc.sync.dma_start(
                        out=dup[64:128, :], in_=sp[hh * 64 : hh * 64 + 64, :]
                    )
                    nc.vector.tensor_mul(out=dup[:], in0=dup[:], in1=cs[:])
                    tiles.append(dup)
                out.append(tiles)
            return out

        et_tiles = {}

        def dots(h, q2, k2):
            ets = []
            for i in range(MC_):
                ps = psA.tile([128, N], F32, tag="ps")
                for hlf in range(2):
                    nc.tensor.matmul(
                        ps[:, hlf * 512 : (hlf + 1) * 512],
                        lhsT=k2[:, i * 128 : (i + 1) * 128],
                        rhs=q2[:, hlf * 512 : (hlf + 1) * 512],
                        start=True,
                        stop=True,
                    )
                et = etp.tile([128, N], BF16, tag="et")
                nc.scalar.activation(et[:], ps[:], AF.Exp, scale=SCALE)
                ets.append(et)
            et_tiles[h] = ets

        posb_state = {}
        ll_state = {}

        def stage2(h):
            ets = et_tiles.pop(h)
            even = h % 2 == 0
            if even:
                ll = llp.tile([33, N], F32, tag="ll")
                lcopies = []
                ll_state[h // 2] = (ll, lcopies)
            else:
                ll, lcopies = ll_state.pop(h // 2)
            lrow = 0 if even else 32
            po_sb = reclp.tile([64, N], BF16, tag="posb")
            for f in range(2):
                po = psO.tile([128, 512], F32, tag="oun")
                for i in range(MC_):
                    nc.tensor.matmul(
                        po[0:65, 0:512],
                        lhsT=vp[:, i, h * 65 : h * 65 + 65],
                        rhs=ets[i][:, f * 512 : (f + 1) * 512],
                        start=(i == 0),
                        stop=(i == MC_ - 1),
                    )
                nc.vector.tensor_copy(
                    out=po_sb[:, f * 512 : (f + 1) * 512], in_=po[0:64, 0:512]
                )
                lcopies.append(
                    nc.vector.tensor_copy(
                        out=ll[lrow : lrow + 1, f * 512 : (f + 1) * 512],
                        in_=po[64:65, 0:512],
                    )
                )
            posb_state[h] = po_sb
            if not even:
                # one approx reciprocal per n-half covers both heads' L
                # rows (partitions 1..31 hold garbage, unread; the op runs
                # at partition base 0 -- custom-DVE ops misbehave at
                # non-zero bases).  Splitting by half lets the tail's
                # recip->bounce->normalize chain start after the f0 drains
                # instead of after the whole pair.  Custom-DVE accesses
                # are invisible to the tile scheduler: order each recip
                # after its L copies explicitly, and make the bounce DMAs
                # in stage2_fin wait on it.
                ris = []
                for f in range(2):
                    ri = nc.vector.reciprocal_approx_fast(
                        out=ll[0:33, f * 512 : (f + 1) * 512],
                        in_=ll[0:33, f * 512 : (f + 1) * 512],
                    )
                    for ci in (lcopies[0 + f], lcopies[2 + f]):
                        _add_dep_helper(
                            ri.ins, ci.ins, sync=True,
                            reason="recip after L-row drains",
                        )
                    ris.append(ri)
                return (ll, ris)
            return None

        def stage2_fin(h, llri):
            """Broadcast 1/L via a DRAM bounce, then normalize (per half)."""
            ll, ris = llri
            po_sb = posb_state.pop(h)
            lrow = 0 if h % 2 == 0 else 32
            prow = (h % 2) * 64
            drs = drsp.tile([1, N], F32, tag="drs")
            lbc = lbcp.tile([64, N], F32, tag="lbc")
            for f in range(2):
                sl = slice(f * 512, (f + 1) * 512)
                di = nc.sync.dma_start(out=drs[0:1, sl], in_=ll[lrow : lrow + 1, sl])
                _add_dep_helper(
                    di.ins, ris[f].ins, sync=True, reason="bounce after recip"
                )
                nc.sync.dma_start(out=lbc[:, sl], in_=bcast(drs[0:1, sl], 64))
                nc.vector.tensor_mul(
                    out=ot_sb[prow : prow + 64, h // 2, sl],
                    in0=po_sb[:, sl],
                    in1=lbc[:, sl],
                )

        # ---- Phases B/C/D interleaved -------------------------------
        # The next pair's qk MATMULS are emitted before dots (so the PE
        # feeds ScalarE early), but their Exp/Ln ACTs are emitted after
        # dots' first-head exps: ScalarE executes its queue in order, and
        # softplus has a full window of slack while dots exps do not.
        ps0 = qk_mms(0)
        v_proj(7)
        q0, k0 = qk_acts(0, ps0)
        dots(0, q0[0], k0[0])
        nxt = qk_acts(1, qk_mms(1))
        dots(1, q0[1], k0[1])
        del ps0

        for j in range(1, EC_Q):
            qj, kj = nxt
            dots(2 * j, qj[0], kj[0])
            # qk(j+1) is emitted after dots(2j) on BOTH engines: the PE
            # runs dots first (feeding ScalarE's exp stream immediately)
            # and ScalarE runs those exps before the j+1 softplus, which
            # matches the shared psA buffer rotation (no deadlock).
            if j + 1 < EC_Q:
                nxt = qk_acts(j + 1, qk_mms(j + 1))
            stage2(2 * j - 2)
            dots(2 * j + 1, qj[1], kj[1])
            ll = stage2(2 * j - 1)
            stage2_fin(2 * j - 2, ll)
            stage2_fin(2 * j - 1, ll)
        stage2(14)
        ll = stage2(15)
        stage2_fin(14, ll)
        stage2_fin(15, ll)

        # ---- Phase F: output projection -----------------------------
        for c in range(NC_):
            ps = psA.tile([128, N], F32, tag="ps")
            for t in range(DT_):
                for hlf in range(2):
                    nc.tensor.matmul(
                        ps[:, hlf * 512 : (hlf + 1) * 512],
                        lhsT=ot_sb[:, t, c * 128 : (c + 1) * 128],
                        rhs=wout_sb[:, t, hlf * 512 : (hlf + 1) * 512],
                        start=(t == 0),
                        stop=(t == DT_ - 1),
                    )
            o_t = outp.tile([128, D], F32, tag="of")
            nc.vector.tensor_add(out=o_t[:], in0=ps[:], in1=bout_bc[:])
            nc.sync.dma_start(out=out_d[c * 128 : (c + 1) * 128, :], in_=o_t[:])


_NC_CACHE = {}


def _get_nc():
    if "nc" not in _NC_CACHE:
        _bacc_mod.get_activation_tables = _patched_tables
        nc = bacc.Bacc(
            "TRN2",
            target_bir_lowering=False,
            debug=False,
            enable_asserts=False,
            num_devices=8,
        )
        with tile.TileContext(nc) as tc:
            _emit(tc)
        nc.compile()
        _NC_CACHE["nc"] = nc
    return _NC_CACHE["nc"]


def _trace_ok():
    try:
        from antenv.axon_hooks import get_axon_ntff_profile_hook

        return get_axon_ntff_profile_hook() is not None
    except Exception:
        return False


def kernel(**inputs):
    bf = ml_dtypes.bfloat16
    x = np.ascontiguousarray(np.asarray(inputs["x"], dtype=np.float32))
    freqs = np.asarray(inputs["freqs"], dtype=np.float32)[0]
    fbias = np.asarray(inputs["bias"], dtype=np.float32)[0]
    g = np.asarray(inputs["ln_gamma"], dtype=np.float32)
    be = np.asarray(inputs["ln_beta"], dtype=np.float32)
    w_qk = np.asarray(inputs["w_qk"], dtype=np.float32)
    w_v = np.asarray(inputs["w_v"], dtype=np.float32)
    w_out = np.asarray(inputs["w_out"], dtype=np.float32)
    b_out = np.asarray(inputs["b_out"], dtype=np.float32)

    wqk_s = np.ascontiguousarray((w_qk * g[:, None]).astype(bf))
    wv_s = np.ascontiguousarray((w_v * g[:, None]).astype(bf))
    wout_b = np.ascontiguousarray(w_out.astype(bf))
    qb = be @ w_qk
    qbias = np.ascontiguousarray(qb.reshape(16, 128).T.astype(np.float32))
    # v-path beta term commutes through the attention average into a
    # constant output offset: b_out' = b_out + (beta @ w_v) @ w_out.
    bout = np.ascontiguousarray(
        (b_out + (be @ w_v) @ w_out)[None, :].astype(np.float32)
    )
    csq = np.ascontiguousarray(
        np.concatenate([np.cos(freqs).T, np.sin(freqs).T], axis=0).astype(bf)
    )
    fb = freqs + fbias
    csk = np.ascontiguousarray(
        np.concatenate([np.cos(fb).T, np.sin(fb).T], axis=0).astype(bf)
    )

    shared = dict(
        wqk=wqk_s, wv=wv_s, wout=wout_b, csq=csq, csk=csk,
        qbias=qbias, bout=bout,
    )
    in_maps = [dict(x=np.ascontiguousarray(x[i]), **shared) for i in range(B)]

    nc = _get_nc()
    want_trace = bool(int(os.environ.get("KERNEL_TRACE", "0")))
    res = run_bass_kernel_spmd(
        nc,
        in_maps,
        core_ids=list(range(B)),
        trace=want_trace and _trace_ok(),
    )
    out = np.stack([res.results[i]["out"] for i in range(B)], axis=0)
    if getattr(res, "exec_time_ns", None):
        kernel.last_exec_time_ns = res.exec_time_ns
    kernel.last_results = res
    return out


# revision 24
# speedup vs baseline: 1.1775x; 1.1775x over previous
"""Fused attention kernel for TRN2, data-parallel over 8 NeuronCores.

Problem: LN -> qk/v projections -> softplus-polar embedding -> attention
-> output projection.  B=8 batch elements are sharded one-per-core; each
core runs an identical single-core program (no collectives).

Layout strategy (per core, N=1024 tokens, D=1024, H=16 heads, DH=64):
  - LN in [n, d] layout (free-axis reductions, bn_stats), PE-transpose
    via a bf16 identity matmul to xnT [d, n] bf16.
  - q/k produced TRANSPOSED ([e, n]) with w_qk tiles as the stationary
    operand; v produced natural ([n, e]) with xnT as stationary.
  - polar: softplus = Ln(1+Exp(x)) on ScalarE (Exp in-place on PSUM; ACT
    ops batched [Exp,Exp,Ln,Ln] per step), the 64-row head slice is
    duplicated into both partition halves by two SBUF->SBUF DMAs, then
    one DVE multiply against a [cosT; sinT] table.
  - attention entirely in transposed layout: S^T = k2t.T @ q2t (K=128,
    single k-tile), exp on ScalarE (scale=DH^-0.5 fused) from PSUM,
    O_un^T = V'.T @ E^T in two n-halves ([65, 512] psum, 1 bank each);
    V' carries a ones-column at col 64 so the softmax denominator L
    lands on psum row 64 for every head.
  - normalize: per half, DVE drains psum rows 0:64 -> po_sb (bf16) and
    row 64 -> the pair tile llp (even head at partition 64, odd head at
    partition 96 -- engine writes must start at partition 0/32/64/96).
    One reciprocal_approx_fast over llp[64:97] covers both heads; a DRAM
    round trip broadcasts each 1/L row to [128, N] and one DVE multiply
    per head writes the normalized O^T into ot_sb.
  - ScalarE activation tables: Exp and Ln are forced into the shared
    `natural_log_exp_and_others` set (see _patched_tables) so the main
    loop never reloads activation tables.
  - final GEMM: lhsT = O^T tiles, rhs = w_out; b_out enters as a DVE
    broadcast add during the PSUM drain (no K=1 bias matmuls).

ln_gamma is folded into w_qk/w_v rows on the host; ln_beta enters as an
ACT bias ([128,1] per e-chunk) for q/k; the v-path beta term (beta@w_v)
commutes through attention (softmax rows sum to 1) and is folded into
b_out on the host: b_out' = b_out + (beta @ w_v) @ w_out.
"""

import os

import ml_dtypes
import numpy as np

import concourse.bass as bass
import concourse.tile as tile
from concourse import bacc, mybir
from concourse.bass_utils import run_bass_kernel_spmd
from concourse.bass import _add_dep_helper
from concourse.masks import make_identity

# ---- force Exp and Ln into one activation-table set -----------------------
# The default greedy assignment puts Exp in `exp_and_others` and Ln in
# `natural_log`, inserting a ~1.3us ACT_TABLE_LOAD at every Exp<->Ln
# transition (2 per main-loop step).  Emptying those two sets (names and
# positions preserved, so act_func_set_id indices stay valid) makes both
# resolve to `natural_log_exp_and_others`.
import concourse.hw_specs as _hw_specs
import concourse.bacc as _bacc_mod

_orig_get_tables = _hw_specs.get_activation_tables


def _patched_tables(arch):
    out = {}
    for name, fns in _orig_get_tables(arch).items():
        if name in ("exp_and_others", "natural_log"):
            out[name] = set()
        else:
            out[name] = fns
    return out


F32 = mybir.dt.float32
BF16 = mybir.dt.bfloat16
AF = mybir.ActivationFunctionType
ALU = mybir.AluOpType

B, N, D, H, DH = 8, 1024, 1024, 16, 64
NC_, DT_, EC_Q, MC_ = 8, 8, 8, 8  # n-chunks, d-tiles, q e-chunks, m-tiles
SCALE = DH ** -0.5


def _emit(tc):
    nc = tc.nc

    x_d = nc.dram_tensor("x", [N, D], F32, kind="ExternalInput").ap()
    wqk_d = nc.dram_tensor("wqk", [D, 2 * H * DH], BF16, kind="ExternalInput").ap()
    wv_d = nc.dram_tensor("wv", [D, H * DH], BF16, kind="ExternalInput").ap()
    wout_d = nc.dram_tensor("wout", [H * DH, D], BF16, kind="ExternalInput").ap()
    csq_d = nc.dram_tensor("csq", [128, N], BF16, kind="ExternalInput").ap()
    csk_d = nc.dram_tensor("csk", [128, N], BF16, kind="ExternalInput").ap()
    qbias_d = nc.dram_tensor("qbias", [128, 16], F32, kind="ExternalInput").ap()
    bout_d = nc.dram_tensor("bout", [1, D], F32, kind="ExternalInput").ap()
    out_d = nc.dram_tensor("out", [N, D], F32, kind="ExternalOutput").ap()

    def bcast(ap_1xN, parts=128):
        return bass.AP(
            tensor=ap_1xN.tensor, offset=ap_1xN.offset, ap=[[0, parts]] + ap_1xN.ap[1:]
        )

    with (
        tc.tile_pool(name="const", bufs=1) as const,
        tc.tile_pool(name="xin", bufs=4) as xin,
        tc.tile_pool(name="ln", bufs=3) as ln,
        tc.tile_pool(name="xnbfp", bufs=3) as xnbfp,
        tc.tile_pool(name="wqs", bufs=2) as wqs,
        tc.tile_pool(name="spp", bufs=2) as spp,
        tc.tile_pool(name="q2p", bufs=4) as q2p,
        tc.tile_pool(name="k2p", bufs=4) as k2p,
        tc.tile_pool(name="etp", bufs=18) as etp,
        tc.tile_pool(name="reclp", bufs=3) as reclp,
        tc.tile_pool(name="llp", bufs=2) as llp,
        tc.tile_pool(name="lbcp", bufs=3) as lbcp,
        tc.tile_pool(name="drsp", bufs=4, space="DRAM") as drsp,
        tc.tile_pool(name="outp", bufs=2) as outp,
        tc.tile_pool(name="psA", bufs=3, space="PSUM") as psA,
        tc.tile_pool(name="psO", bufs=2, space="PSUM") as psO,
    ):
        # ---- resident constants -------------------------------------
        # x prefetch first: the LN chain is the critical path at startup.
        x_tiles = {}

        def fetch_x(c):
            if c < NC_ and c not in x_tiles:
                t = xin.tile([128, D], F32, tag="x")
                nc.sync.dma_start(out=t[:], in_=x_d[c * 128 : (c + 1) * 128, :])
                x_tiles[c] = t

        for c in range(3):
            fetch_x(c)

        # wv arrives piecewise: v-proj's t-th matmul only needs piece t,
        # so early pieces unblock chunk-0 v work while the rest stream in.
        # The scalar queue carries no DMAs (it must reach the LN Sqrts
        # quickly); wout is issued at the end of Phase A emission.
        wv_sb = const.tile([128, DT_, 1024], BF16, tag="wv")
        wv_r = wv_d.rearrange("(t p) e -> p t e", p=128)
        nc.gpsimd.dma_start(out=wv_sb[:, 0:4, :], in_=wv_r[:, 0:4, :])
        nc.sync.dma_start(out=wv_sb[:, 4:8, :], in_=wv_r[:, 4:8, :])
        csq_sb = const.tile([128, N], BF16, tag="csq")
        nc.sync.dma_start(out=csq_sb[:], in_=csq_d)
        csk_sb = const.tile([128, N], BF16, tag="csk")
        nc.sync.dma_start(out=csk_sb[:], in_=csk_d)
        qbias_sb = const.tile([128, 16], F32, tag="qbias")
        nc.sync.dma_start(out=qbias_sb[:], in_=qbias_d)
        bout_bc = const.tile([128, 1024], F32, tag="boutbc")
        nc.gpsimd.dma_start(out=bout_bc[:], in_=bcast(bout_d))
        eps_sb = const.tile([128, 1], F32, tag="eps")
        nc.vector.memset(eps_sb[:], 1e-5)
        ident = const.tile([128, 128], BF16, tag="ident")
        make_identity(nc, ident[:])

        xnT = const.tile([128, DT_, N], BF16, tag="xnT")
        # V' per m-chunk: [m-part, chunk, head*65]; col 64 of each head
        # block is the ones-column (softmax denominator row).
        vp = const.tile([128, MC_, H * 65], BF16, tag="vp")
        nc.gpsimd.memset(
            vp.rearrange("p m (h w) -> p m h w", w=65)[:, :, :, 64:65], 1.0
        )
        ot_sb = const.tile([128, DT_, N], BF16, tag="otsb")

        # ---- Phase A: layernorm + PE transpose + v projection -------
        ln_state = {}
        xnbf_tiles = {}

        def ln_stats(c):
            if c >= NC_ or c in ln_state:
                return
            x_t = x_tiles[c]
            st = ln.tile([128, 2, 6], F32, tag="st")
            for s in range(2):
                nc.vector.bn_stats(out=st[:, s, :], in_=x_t[:, s * 512 : (s + 1) * 512])
            mv = ln.tile([128, 2], F32, tag="mv")
            nc.vector.bn_aggr(out=mv[:], in_=st[:])
            rsig = ln.tile([128, 1], F32, tag="rsig")
            # rsqrt via exp(-0.5*ln(var+eps)): keeps ScalarE on the one
            # shared exp/ln table set (no Sqrt-table load mid-kernel).
            nc.scalar.activation(rsig[:], mv[:, 1:2], AF.Ln, bias=eps_sb[:])
            nc.scalar.activation(rsig[:], rsig[:], AF.Exp, scale=-0.5)
            ln_state[c] = (mv, rsig)

        def ln_ts(c):
            if c >= NC_ or c in xnbf_tiles:
                return
            mv, rsig = ln_state.pop(c)
            xnbf = xnbfp.tile([128, D], BF16, tag="xnbf")
            nc.vector.tensor_scalar(
                out=xnbf[:],
                in0=x_tiles[c],
                scalar1=mv[:, 0:1],
                scalar2=rsig[:],
                op0=ALU.subtract,
                op1=ALU.mult,
            )
            xnbf_tiles[c] = xnbf

        def v_proj(c):
            psv = psA.tile([128, N], F32, tag="ps")
            for t in range(DT_):
                for hlf in range(2):
                    nc.tensor.matmul(
                        psv[:, hlf * 512 : (hlf + 1) * 512],
                        lhsT=xnT[:, t, c * 128 : (c + 1) * 128],
                        rhs=wv_sb[:, t, hlf * 512 : (hlf + 1) * 512],
                        start=(t == 0),
                        stop=(t == DT_ - 1),
                    )
            nc.vector.tensor_copy(
                out=vp.rearrange("p m (h w) -> p m h w", w=65)[:, c, :, 0:64],
                in_=psv.rearrange("p (h w) -> p h w", w=64),
            )

        ln_stats(0)
        ln_ts(0)
        ln_stats(1)
        ln_ts(1)
        ln_stats(2)
        ln_ts(2)
        # v projection runs one chunk behind the transposes so the xnT
        # psum->SBUF drain latency never sits on the PE critical path;
        # v(7) is emitted after qk(0)'s matmuls to fill the PE while the
        # first softplus/polar chain runs.
        for c in range(NC_):
            xnbf = xnbf_tiles.pop(c)
            pst = psA.tile([128, N], F32, tag="ps")
            for t in range(DT_):
                nc.tensor.matmul(
                    pst[:, t * 128 : (t + 1) * 128],
                    lhsT=xnbf[:, t * 128 : (t + 1) * 128],
                    rhs=ident[:],
                    start=True,
                    stop=True,
                )
            nc.vector.tensor_copy(
                out=xnT[:, :, c * 128 : (c + 1) * 128],
                in_=pst.rearrange("p (t n) -> p t n", n=128),
            )
            ln_ts(c + 2)
            fetch_x(c + 3)
            if c > 1:
                v_proj(c - 2)
            ln_stats(c + 3)

        wout_sb = const.tile([128, DT_, 1024], BF16, tag="wout")
        wout_r = wout_d.rearrange("(t p) e -> p t e", p=128)
        nc.gpsimd.dma_start(out=wout_sb[:, 0:4, :], in_=wout_r[:, 0:4, :])
        nc.sync.dma_start(out=wout_sb[:, 4:8, :], in_=wout_r[:, 4:8, :])

        # ---- helpers ------------------------------------------------
        def qk_mms(j):
            psqk = []
            for is_q in (True, False):
                ecol = j * 128 if is_q else 1024 + j * 128
                wt = wqs.tile([128, DT_, 128], BF16, tag="wt")
                nc.sync.dma_start(
                    out=wt[:],
                    in_=wqk_d.rearrange("(t p) e -> p t e", p=128)[
                        :, :, ecol : ecol + 128
                    ],
                )
                ps = psA.tile([128, N], F32, tag="ps")
                for t in range(DT_):
                    for hlf in range(2):
                        nc.tensor.matmul(
                            ps[:, hlf * 512 : (hlf + 1) * 512],
                            lhsT=wt[:, t, :],
                            rhs=xnT[:, t, hlf * 512 : (hlf + 1) * 512],
                            start=(t == 0),
                            stop=(t == DT_ - 1),
                        )
                psqk.append(ps)
            return psqk

        def qk_acts(j, psqk):
            # Exp/Ln share one table set, so interleave per operand:
            # Exp_q,Ln_q run as soon as q's 8 matmuls land (not after all
            # 16), and the q psum frees a rotation slot earlier.
            sps = []
            for is_q, ps in zip((True, False), psqk):
                bcol = j if is_q else 8 + j
                nc.scalar.activation(
                    ps[:], ps[:], AF.Exp, bias=qbias_sb[:, bcol : bcol + 1]
                )
                sp = spp.tile([128, N], BF16, tag="sp")
                nc.scalar.activation(sp[:], ps[:], AF.Ln, bias=1.0)
                sps.append(sp)
            out = []
            for is_q, sp in zip((True, False), sps):
                pool = q2p if is_q else k2p
                cs = csq_sb if is_q else csk_sb
                tiles = []
                for hh in range(2):
                    dup = pool.tile([128, N], BF16, tag="d")
                    nc.sync.dma_start(
                        out=dup[0:64, :], in_=sp[hh * 64 : hh * 64 + 64, :]
                    )
                    nc.sync.dma_start(
                        out=dup[64:128, :], in_=sp[hh * 64 : hh * 64 + 64, :]
                    )
                    nc.vector.tensor_mul(out=dup[:], in0=dup[:], in1=cs[:])
                    tiles.append(dup)
                out.append(tiles)
            return out

        et_tiles = {}

        def dots(h, q2, k2):
            ets = []
            for i in range(MC_):
                ps = psA.tile([128, N], F32, tag="ps")
                for hlf in range(2):
                    nc.tensor.matmul(
                        ps[:, hlf * 512 : (hlf + 1) * 512],
                        lhsT=k2[:, i * 128 : (i + 1) * 128],
                        rhs=q2[:, hlf * 512 : (hlf + 1) * 512],
                        start=True,
                        stop=True,
                    )
                et = etp.tile([128, N], BF16, tag="et")
                nc.scalar.activation(et[:], ps[:], AF.Exp, scale=SCALE)
                ets.append(et)
            et_tiles[h] = ets

        posb_state = {}
        ll_state = {}

        def stage2(h):
            ets = et_tiles.pop(h)
            even = h % 2 == 0
            if even:
                ll = llp.tile([33, N], F32, tag="ll")
                lcopies = []
                ll_state[h // 2] = (ll, lcopies)
            else:
                ll, lcopies = ll_state.pop(h // 2)
            lrow = 0 if even else 32
            po_sb = reclp.tile([64, N], BF16, tag="posb")
            for f in range(2):
                po = psO.tile([128, 512], F32, tag="oun")
                for i in range(MC_):
                    nc.tensor.matmul(
                        po[0:65, 0:512],
                        lhsT=vp[:, i, h * 65 : h * 65 + 65],
                        rhs=ets[i][:, f * 512 : (f + 1) * 512],
                        start=(i == 0),
                        stop=(i == MC_ - 1),
                    )
                nc.vector.tensor_copy(
                    out=po_sb[:, f * 512 : (f + 1) * 512], in_=po[0:64, 0:512]
                )
                lcopies.append(
                    nc.vector.tensor_copy(
                        out=ll[lrow : lrow + 1, f * 512 : (f + 1) * 512],
                        in_=po[64:65, 0:512],
                    )
                )
            posb_state[h] = po_sb
            if not even:
                # one approx reciprocal per n-half covers both heads' L
                # rows (partitions 1..31 hold garbage, unread; the op runs
                # at partition base 0 -- custom-DVE ops misbehave at
                # non-zero bases).  Splitting by half lets the tail's
                # recip->bounce->normalize chain start after the f0 drains
                # instead of after the whole pair.  Custom-DVE accesses
                # are invisible to the tile scheduler: order each recip
                # after its L copies explicitly, and make the bounce DMAs
                # in stage2_fin wait on it.
                ris = []
                for f in range(2):
                    ri = nc.vector.reciprocal_approx_fast(
                        out=ll[0:33, f * 512 : (f + 1) * 512],
                        in_=ll[0:33, f * 512 : (f + 1) * 512],
                    )
                    for ci in (lcopies[0 + f], lcopies[2 + f]):
                        _add_dep_helper(
                            ri.ins, ci.ins, sync=True,
                            reason="recip after L-row drains",
                        )
                    ris.append(ri)
                return (ll, ris)
            return None

        def stage2_fin(h, llri):
            """Broadcast 1/L via a DRAM bounce, then normalize (per half)."""
            ll, ris = llri
            po_sb = posb_state.pop(h)
            lrow = 0 if h % 2 == 0 else 32
            prow = (h % 2) * 64
            drs = drsp.tile([1, N], F32, tag="drs")
            lbc = lbcp.tile([64, N], F32, tag="lbc")
            for f in range(2):
                sl = slice(f * 512, (f + 1) * 512)
                di = nc.sync.dma_start(out=drs[0:1, sl], in_=ll[lrow : lrow + 1, sl])
                _add_dep_helper(
                    di.ins, ris[f].ins, sync=True, reason="bounce after recip"
                )
                nc.sync.dma_start(out=lbc[:, sl], in_=bcast(drs[0:1, sl], 64))
                nc.vector.tensor_mul(
                    out=ot_sb[prow : prow + 64, h // 2, sl],
                    in0=po_sb[:, sl],
                    in1=lbc[:, sl],
                )

        # ---- Phases B/C/D interleaved -------------------------------
        # The next pair's qk MATMULS are emitted before dots (so the PE
        # feeds ScalarE early), but their Exp/Ln ACTs are emitted after
        # dots' first-head exps: ScalarE executes its queue in order, and
        # softplus has a full window of slack while dots exps do not.
        ps0 = qk_mms(0)
        v_proj(6)
        q0, k0 = qk_acts(0, ps0)
        v_proj(7)
        dots(0, q0[0], k0[0])
        nxt = qk_acts(1, qk_mms(1))
        dots(1, q0[1], k0[1])
        del ps0

        for j in range(1, EC_Q):
            qj, kj = nxt
            dots(2 * j, qj[0], kj[0])
            # qk(j+1) is emitted after dots(2j) on BOTH engines: the PE
            # runs dots first (feeding ScalarE's exp stream immediately)
            # and ScalarE runs those exps before the j+1 softplus, which
            # matches the shared psA buffer rotation (no deadlock).
            if j + 1 < EC_Q:
                nxt = qk_acts(j + 1, qk_mms(j + 1))
            stage2(2 * j - 2)
            dots(2 * j + 1, qj[1], kj[1])
            ll = stage2(2 * j - 1)
            stage2_fin(2 * j - 2, ll)
            stage2_fin(2 * j - 1, ll)
        stage2(14)
        ll = stage2(15)
        stage2_fin(14, ll)
        stage2_fin(15, ll)

        # ---- Phase F: output projection -----------------------------
        for c in range(NC_):
            ps = psA.tile([128, N], F32, tag="ps")
            for t in range(DT_):
                for hlf in range(2):
                    nc.tensor.matmul(
                        ps[:, hlf * 512 : (hlf + 1) * 512],
                        lhsT=ot_sb[:, t, c * 128 : (c + 1) * 128],
                        rhs=wout_sb[:, t, hlf * 512 : (hlf + 1) * 512],
                        start=(t == 0),
                        stop=(t == DT_ - 1),
                    )
            o_t = outp.tile([128, D], F32, tag="of")
            nc.vector.tensor_add(out=o_t[:], in0=ps[:], in1=bout_bc[:])
            nc.sync.dma_start(out=out_d[c * 128 : (c + 1) * 128, :], in_=o_t[:])


_NC_CACHE = {}


def _get_nc():
    if "nc" not in _NC_CACHE:
        _bacc_mod.get_activation_tables = _patched_tables
        nc = bacc.Bacc(
            "TRN2",
            target_bir_lowering=False,
            debug=False,
            enable_asserts=False,
            num_devices=8,
        )
        with tile.TileContext(nc) as tc:
            _emit(tc)
        nc.compile()
        _NC_CACHE["nc"] = nc
    return _NC_CACHE["nc"]


def _trace_ok():
    try:
        from antenv.axon_hooks import get_axon_ntff_profile_hook

        return get_axon_ntff_profile_hook() is not None
    except Exception:
        return False


def kernel(**inputs):
    bf = ml_dtypes.bfloat16
    x = np.ascontiguousarray(np.asarray(inputs["x"], dtype=np.float32))
    freqs = np.asarray(inputs["freqs"], dtype=np.float32)[0]
    fbias = np.asarray(inputs["bias"], dtype=np.float32)[0]
    g = np.asarray(inputs["ln_gamma"], dtype=np.float32)
    be = np.asarray(inputs["ln_beta"], dtype=np.float32)
    w_qk = np.asarray(inputs["w_qk"], dtype=np.float32)
    w_v = np.asarray(inputs["w_v"], dtype=np.float32)
    w_out = np.asarray(inputs["w_out"], dtype=np.float32)
    b_out = np.asarray(inputs["b_out"], dtype=np.float32)

    wqk_s = np.ascontiguousarray((w_qk * g[:, None]).astype(bf))
    wv_s = np.ascontiguousarray((w_v * g[:, None]).astype(bf))
    wout_b = np.ascontiguousarray(w_out.astype(bf))
    qb = be @ w_qk
    qbias = np.ascontiguousarray(qb.reshape(16, 128).T.astype(np.float32))
    # v-path beta term commutes through the attention average into a
    # constant output offset: b_out' = b_out + (beta @ w_v) @ w_out.
    bout = np.ascontiguousarray(
        (b_out + (be @ w_v) @ w_out)[None, :].astype(np.float32)
    )
    csq = np.ascontiguousarray(
        np.concatenate([np.cos(freqs).T, np.sin(freqs).T], axis=0).astype(bf)
    )
    fb = freqs + fbias
    csk = np.ascontiguousarray(
        np.concatenate([np.cos(fb).T, np.sin(fb).T], axis=0).astype(bf)
    )

    shared = dict(
        wqk=wqk_s, wv=wv_s, wout=wout_b, csq=csq, csk=csk,
        qbias=qbias, bout=bout,
    )
    in_maps = [dict(x=np.ascontiguousarray(x[i]), **shared) for i in range(B)]

    nc = _get_nc()
    want_trace = bool(int(os.environ.get("KERNEL_TRACE", "0")))
    res = run_bass_kernel_spmd(
        nc,
        in_maps,
        core_ids=list(range(B)),
        trace=want_trace and _trace_ok(),
    )
    out = np.stack([res.results[i]["out"] for i in range(B)], axis=0)
    if getattr(res, "exec_time_ns", None):
        kernel.last_exec_time_ns = res.exec_time_ns
    kernel.last_results = res
    return out
